# revision 33
# baseline (speedup 1.0000x reference)
"""GroupedQueryAttention (B=1, T=2048, D=4096, 32 q-heads / 8 kv-heads, hd=128)
on 8 trn2 NeuronCores.

Sharding: kv-head parallel — core c owns kv head c and its 4 query heads.

Design (final):
- 16-bit matmuls: fp16 on projections and the q/k score path (mantissa),
  bf16 on the exp/value path (range: exp(score) can exceed fp16 max).
- Causal attention in transposed [k, q] score layout; softmax without
  max-subtraction (fp32 PSUM exp can't overflow at these score scales);
  denominator accumulated on the vector engine in two interleaved partial
  sums, reduced/broadcast via tiny PE matmuls pipelined two heads behind
  the score loop.
- Chunk-pipelined schedule: per 512-token chunk proj -> attn -> AllGather
  (fp16, issued after the next chunk's DMAs are enqueued so the collective
  ring never blocks compute loads) -> column-parallel wo (y streamed
  per-kt, 4 PSUM accumulators), placed two chunks late to hide collective
  latency.  Dummy warm-up matmuls keep the PE HAM clock at full rate
  through the initial weight load and the final AllGather wait.
- DMA triggers spread across sync/scalar/gpsimd engine queues; weights
  host-prepacked so every SBUF partition's data is DRAM-contiguous.
"""
import sys

sys.path.insert(0, "/opt/trn_rl_repo")

import numpy as np

import concourse.bacc as bacc
import concourse.tile as tile
from concourse import mybir
from concourse.bass_utils import run_bass_kernel_spmd
from concourse.masks import make_identity

N_CORES = 8
T = 2048
DIM = 4096
HD = 128
NH = 32
NKV = 8
NREP = NH // NKV  # 4 query heads per core
NCHUNK = T // 512  # 4 chunks of 512 along T
NKT = DIM // 128  # 32 contraction tiles for the projections
NTT = T // 128  # 16 row tiles for the wo matmul
F32 = mybir.dt.float32
FP16 = mybir.dt.float16
BF16 = mybir.dt.bfloat16
SCALE = 1.0 / float(np.sqrt(HD))

_cached = {}


def _build_kernel():
    if "nc" in _cached:
        return _cached["nc"]

    nc = bacc.Bacc("TRN2", target_bir_lowering=False)

    xT = nc.dram_tensor("xT", [DIM, T], FP16, kind="ExternalInput")
    cos2 = nc.dram_tensor("cos2", [128, T], FP16, kind="ExternalInput")
    sin2 = nc.dram_tensor("sin2", [128, T], FP16, kind="ExternalInput")
    masks = nc.dram_tensor("masks", [128, 4 * 512], BF16, kind="ExternalInput")
    # weights pre-packed on host: [128, n*m] with partition-contiguous rows
    wq_p = nc.dram_tensor("wq_p", [128, NKT * NREP * HD], FP16, kind="ExternalInput")
    wk_p = nc.dram_tensor("wk_p", [128, NKT * HD], FP16, kind="ExternalInput")
    wv_p = nc.dram_tensor("wv_p", [128, NKT * HD], FP16, kind="ExternalInput")
    wo_p = nc.dram_tensor("wo_p", [128, NKT * NREP * HD], FP16, kind="ExternalInput")
    out = nc.dram_tensor("out", [T, NREP * HD], F32, kind="ExternalOutput")

    y_in = [
        nc.dram_tensor(f"y_in{qc}", [NREP * HD, 512], FP16, kind="Internal")
        for qc in range(NCHUNK)
    ]
    y_all = [
        nc.dram_tensor(
            f"y_all{qc}", [DIM, 512], FP16, kind="Internal", addr_space="Shared"
        )
        for qc in range(NCHUNK)
    ]

    with tile.TileContext(nc) as tc:
        with (
            tc.tile_pool(name="consts", bufs=1) as consts,
            tc.tile_pool(name="weights", bufs=1) as weights,
            tc.tile_pool(name="acts", bufs=1) as acts,
            tc.tile_pool(name="stream", bufs=12) as stream,
            tc.tile_pool(name="ystream", bufs=10) as ystream,
            tc.tile_pool(name="work", bufs=2) as work,
            tc.tile_pool(name="lrec", bufs=2) as lrec,
            tc.tile_pool(name="expp", bufs=8) as expp,
            tc.tile_pool(name="outp", bufs=4) as outp,
            tc.tile_pool(name="psum", bufs=7, space="PSUM") as psum,
            tc.tile_pool(name="psumv", bufs=1, space="PSUM") as psumv,
        ):
            # ---------- resident weights, kt-sliced so proj(0) starts early ----
            wq_r = wq_p.rearrange("p (n m) -> p n m", n=NKT)
            wq_sbs = []
            wq_sbs.append(weights.tile([128, 8, NREP * HD], FP16, tag="wq0", name="wq_t0"))
            wk_sb = weights.tile([128, NKT, HD], FP16, tag="wk")
            wv_sb = weights.tile([128, NKT, HD], FP16, tag="wv")
            wk_r = wk_p.rearrange("p (n m) -> p n m", n=NKT)
            wv_r = wv_p.rearrange("p (n m) -> p n m", n=NKT)
            for j in range(2):
                nc.gpsimd.dma_start(
                    out=wq_sbs[0][:, 4 * j:4 * (j + 1), :],
                    in_=wq_r[:, 4 * j:4 * (j + 1), :],
                )
            nc.gpsimd.dma_start(out=wk_sb, in_=wk_r)
            nc.gpsimd.dma_start(out=wv_sb, in_=wv_r)
            for s in range(1, 4):
                wq_sbs.append(
                    weights.tile(
                        [128, 8, NREP * HD], FP16, tag=f"wq{s}", name=f"wq_t{s}"
                    )
                )
                nc.gpsimd.dma_start(
                    out=wq_sbs[s], in_=wq_r[:, 8 * s:8 * (s + 1), :]
                )
            wo_sb = weights.tile([128, NKT, NREP * HD], FP16, tag="wo")

            # ---------- constants (needed only once rope starts) ----------
            cos_sb = consts.tile([128, T], FP16, tag="cos")
            nc.scalar.dma_start(out=cos_sb, in_=cos2[:, :])
            sin_sb = consts.tile([128, T], FP16, tag="sin")
            nc.scalar.dma_start(out=sin_sb, in_=sin2[:, :])
            mask_sb = consts.tile([128, 4 * 512], BF16, tag="mask")
            nc.scalar.dma_start(out=mask_sb, in_=masks[:, :])
            ones_col = consts.tile([128, 1], BF16, tag="onesc")
            nc.vector.memset(ones_col, 1.0)
            ones_row = consts.tile([1, 128], BF16, tag="onesr")
            nc.vector.memset(ones_row, 1.0)
            ident = consts.tile([128, 128], BF16, tag="ident")
            make_identity(nc, ident)

            warm_sb = consts.tile([128, 64], FP16, tag="warm")
            nc.vector.memset(warm_sb, 0.0)

            def pe_warm(n):
                """Dummy matmuls to keep the PE HAM clock warm during waits."""
                w_ps = psumv.tile([64, 64], F32, tag="vtbank", name=f"wp{pe_warm.i}")
                pe_warm.i += 1
                for i in range(n):
                    nc.tensor.matmul(
                        w_ps, lhsT=warm_sb[:, 0:64], rhs=warm_sb,
                        start=(i == 0), stop=(i == n - 1),
                        skip_group_check=True,
                    )
            pe_warm.i = 0

            pe_warm(500)

            # activations that live through the attention phase
            qT_sb = acts.tile([128, NREP, T], FP16, tag="qt")
            kT_sb = acts.tile([128, T], FP16, tag="kt")
            vkd_sb = acts.tile([128, NTT, HD], BF16, tag="vkd")

            def proj_chunk(qc):
                """QKV projections + rope for token chunk qc."""
                cs = slice(512 * qc, 512 * (qc + 1))
                q_ps = [
                    psum.tile([128, 512], F32, tag="bank", name=f"qps{qc}_{h}")
                    for h in range(NREP)
                ]
                k_ps = psum.tile([128, 512], F32, tag="bank", name=f"kps{qc}")
                v_ps = psum.tile([128, 512], F32, tag="bank", name=f"vps{qc}")
                for kt in range(NKT):
                    xt = stream.tile([128, 512], FP16, tag="xt")
                    nc.sync.dma_start(
                        out=xt, in_=xT[128 * kt:128 * (kt + 1), cs]
                    )
                    st = kt == 0
                    sp = kt == NKT - 1
                    for h in range(NREP):
                        nc.tensor.matmul(
                            q_ps[h],
                            lhsT=wq_sbs[kt // 8][:, kt % 8, 128 * h:128 * (h + 1)],
                            rhs=xt,
                            start=st,
                            stop=sp,
                        )
                    nc.tensor.matmul(
                        k_ps, lhsT=wk_sb[:, kt, :], rhs=xt, start=st, stop=sp
                    )
                    nc.tensor.matmul(
                        v_ps, lhsT=wv_sb[:, kt, :], rhs=xt, start=st, stop=sp
                    )

                # rope: k first (unblocks h=0 scores), then the 4 q heads
                for h in [NREP, 0, 1, 2, 3]:
                    p = q_ps[h] if h < NREP else k_ps
                    dst = qT_sb[:, h, cs] if h < NREP else kT_sb[:, cs]
                    sw = work.tile([128, 512], F32, tag="sw")
                    nc.scalar.copy(sw[0:64, :], p[64:128, :])
                    nc.scalar.copy(sw[64:128, :], p[0:64, :])
                    rtmp = work.tile([128, 512], F32, tag="ropetmp")
                    # dst = p * cos + sw * (+-sin), fp16 conversion on the add
                    nc.vector.tensor_mul(rtmp, p, cos_sb[:, cs])
                    nc.vector.tensor_mul(sw, sw, sin_sb[:, cs])
                    nc.vector.tensor_add(dst, rtmp, sw)

                # v computed in [hd, T] layout; transpose 128x128 blocks to [k, hd]
                v_sb = work.tile([128, 512], BF16, tag="vsb")
                nc.scalar.copy(v_sb, v_ps)
                for s in range(4):
                    vt_ps = psumv.tile(
                        [128, 128], BF16, tag="vtbank", name=f"vt{qc}_{s}"
                    )
                    nc.tensor.transpose(
                        vt_ps, v_sb[:, 128 * s:128 * (s + 1)], ident
                    )
                    nc.scalar.copy(vkd_sb[:, 4 * qc + s, :], vt_ps)

            def attn_span(qc, c0, w, y_in_t):
                """Causal attention for all 4 heads on cols [c0, c0+w).

                The softmax finalize (denominator reduce, reciprocal,
                broadcast, normalize, store) for head h is issued after head
                h+1's score loop so its serial chain overlaps PE work; the
                last head's finalize is returned as a closure the scheduler
                places under later PE work.
                """
                cs = slice(c0, c0 + w)
                nkt = (c0 + w) // 128  # causal k tiles
                pe_warm(100)
                pendA = []
                pendB = []

                def stage_a():
                    # free the PSUM bank, reduce denominator, reciprocal
                    h, yT_ps, l_acc = pendA.pop(0)
                    yT_sb = work.tile([128, 512], F32, tag="ytsb")
                    nc.scalar.copy(yT_sb[:, 0:w], yT_ps[:, 0:w])
                    l_bf = work.tile([128, 512], BF16, tag="lbf")
                    nc.scalar.copy(l_bf[:, 0:w], l_acc[:, 0:w])
                    l_ps = psum.tile([128, 512], F32, tag="bank", name=f"l{c0}{h}")
                    nc.tensor.matmul(
                        l_ps[0:1, 0:w], lhsT=ones_col[:, 0:1], rhs=l_bf[:, 0:w],
                        start=True, stop=True,
                    )
                    recip = lrec.tile([1, 512], F32, tag="recip")
                    nc.vector.reciprocal_approx_fast(recip[:, 0:w], l_ps[0:1, 0:w])
                    recip_bf = lrec.tile([1, 512], BF16, tag="recipbf")
                    nc.scalar.copy(recip_bf[:, 0:w], recip[:, 0:w])
                    pendB.append((h, yT_sb, recip_bf))

                def stage_b():
                    # broadcast 1/l to 128 partitions, normalize, store
                    h, yT_sb, recip_bf = pendB.pop(0)
                    bc_ps = psum.tile([128, 512], F32, tag="bank", name=f"b{c0}{h}")
                    nc.tensor.matmul(
                        bc_ps[:, 0:w], lhsT=ones_row[0:1, :], rhs=recip_bf[0:1, 0:w],
                        start=True, stop=True,
                    )
                    bc_sb = work.tile([128, 512], F32, tag="bc")
                    nc.scalar.copy(bc_sb[:, 0:w], bc_ps[:, 0:w])
                    yn_sb = work.tile([128, 512], FP16, tag="yn")
                    nc.vector.tensor_mul(yn_sb[:, 0:w], yT_sb[:, 0:w], bc_sb[:, 0:w])
                    nc.sync.dma_start(
                        out=y_in_t[128 * h:128 * (h + 1), :], in_=yn_sb[:, 0:w]
                    )

                for h in range(NREP):
                    yT_ps = psum.tile(
                        [128, 512], F32, tag="bank", name=f"yps{c0}_{h}"
                    )
                    l_acc = lrec.tile([128, 512], F32, tag="lacc")
                    nc.vector.memset(l_acc[:, 0:w], 0.0)
                    l_acc2 = lrec.tile([128, 512], F32, tag="lacc2")
                    if nkt > 4:
                        nc.vector.memset(l_acc2[:, 0:w], 0.0)
                    for kt in range(nkt):
                        sT_ps = psum.tile(
                            [128, 512], F32, tag="bank", name=f"sps{c0}_{h}_{kt}"
                        )
                        nc.tensor.matmul(
                            sT_ps[:, 0:w],
                            lhsT=kT_sb[:, 128 * kt:128 * (kt + 1)],
                            rhs=qT_sb[:, h, cs],
                            start=True,
                            stop=True,
                        )
                        e_sb = expp.tile([128, 512], BF16, tag="exp")
                        nc.scalar.activation(
                            e_sb[:, 0:w], sT_ps[:, 0:w],
                            mybir.ActivationFunctionType.Exp,
                            scale=SCALE,
                        )
                        dd = (128 * kt - c0) // 128
                        if dd >= 0:  # diagonal block: zero the k > q half
                            nc.vector.tensor_mul(
                                e_sb[:, 0:w], e_sb[:, 0:w],
                                mask_sb[:, 512 * dd:512 * dd + w]
                            )
                        la = l_acc if (nkt <= 4 or kt % 2 == 0) else l_acc2
                        nc.vector.tensor_add(la[:, 0:w], la[:, 0:w], e_sb[:, 0:w])
                        nc.tensor.matmul(
                            yT_ps[:, 0:w],
                            lhsT=vkd_sb[:, kt, :],
                            rhs=e_sb[:, 0:w],
                            start=(kt == 0),
                            stop=(kt == nkt - 1),
                        )
                    if nkt > 4:
                        nc.vector.tensor_add(
                            l_acc[:, 0:w], l_acc[:, 0:w], l_acc2[:, 0:w]
                        )
                    pendA.append((h, yT_ps, l_acc))
                    if h >= 1:
                        stage_a()  # head h-1, under head h's PE work
                    if h >= 2:
                        stage_b()  # head h-2

                def rest():
                    stage_a()
                    stage_b()
                    stage_b()
                return rest

            def attn_chunk(qc):
                return attn_span(qc, 512 * qc, 512, y_in[qc])
            def gather_span(in_t, out_t):
                nc.gpsimd.collective_compute(
                    "AllGather",
                    mybir.AluOpType.bypass,
                    ins=[in_t[:, :]],
                    outs=[out_t[:, :]],
                    replica_groups=[list(range(N_CORES))],
                )

            def gather_chunk(qc):
                gather_span(y_in[qc], y_all[qc])

            def wo_span(y_all_t, tt0, w):
                """out rows [128*tt0, 128*tt0 + w): y streams per-kt through
                the ystream pool; w//128 output row-tiles accumulate in
                parallel PSUM banks."""
                ntt = w // 128
                y_r = y_all_t.rearrange("(n p) m -> p n m", p=128)
                o_ps = [
                    psum.tile([128, 512], F32, tag="bank", name=f"o{tt0}_{t2}")
                    for t2 in range(ntt)
                ]
                for kt in range(NKT):
                    y_kt = ystream.tile([128, 512], FP16, tag="yk", name=f"yk{tt0}_{kt}")
                    nc.gpsimd.dma_start(out=y_kt[:, 0:w], in_=y_r[:, kt, :])
                    for t2 in range(ntt):
                        nc.tensor.matmul(
                            o_ps[t2],
                            lhsT=y_kt[:, 128 * t2:128 * (t2 + 1)],
                            rhs=wo_sb[:, kt, :],
                            start=(kt == 0),
                            stop=(kt == NKT - 1),
                        )
                for t2 in range(ntt):
                    tt = tt0 + t2
                    o_sb = outp.tile([128, 512], F32, tag="osb")
                    nc.scalar.copy(o_sb, o_ps[t2])
                    nc.sync.dma_start(
                        out=out[128 * tt:128 * (tt + 1), :], in_=o_sb
                    )

            def wo_chunk(qc):
                wo_span(y_all[qc], 4 * qc, 512)

            # ---------- chunk-pipelined schedule ----------
            # wo(qc) is issued two chunks late so the AllGather latency is
            # covered by proj/attn of the following chunks.
            proj_chunk(0)
            fin0 = attn_chunk(0)
            wo_r = wo_p.rearrange("p (n m) -> p n m", n=NKT)
            for s in range(4):
                nc.gpsimd.dma_start(
                    out=wo_sb[:, 8 * s:8 * (s + 1), :],
                    in_=wo_r[:, 8 * s:8 * (s + 1), :],
                )
            proj_chunk(1)
            fin0()
            gather_chunk(0)
            fin1 = attn_chunk(1)
            proj_chunk(2)
            fin1()
            gather_chunk(1)
            wo_chunk(0)
            fin2 = attn_chunk(2)
            proj_chunk(3)
            fin2()
            gather_chunk(2)
            wo_chunk(1)
            fin3 = attn_chunk(3)
            fin3()
            wo_chunk(2)
            gather_chunk(3)
            pe_warm(800)
            wo_chunk(3)

    nc.compile()
    _cached["nc"] = nc
    return nc


def _build_in_maps(inputs):
    return _shard_inputs(**inputs)


def _pack_w(wT, m):
    """[DIM, m] -> [128, NKT*m] with each partition's rows DRAM-contiguous."""
    return np.ascontiguousarray(
        wT.reshape(NKT, 128, m).transpose(1, 0, 2).reshape(128, NKT * m)
    )


def _shard_inputs(x, cos, sin, wq, wk, wv, wo, start_pos):
    import ml_dtypes

    bf16 = ml_dtypes.bfloat16
    x = np.asarray(x, dtype=np.float32)
    cos = np.asarray(cos, dtype=np.float32)
    sin = np.asarray(sin, dtype=np.float32)
    wq = np.asarray(wq, dtype=np.float32)
    wk = np.asarray(wk, dtype=np.float32)
    wv = np.asarray(wv, dtype=np.float32)
    wo = np.asarray(wo, dtype=np.float32)
    sp = int(start_pos)

    xT = np.ascontiguousarray(x[0].T).astype(np.float16)  # (DIM, T)
    cosT = np.ascontiguousarray(cos[sp:sp + T].T)  # (64, T)
    sinT = np.ascontiguousarray(sin[sp:sp + T].T)
    cos2 = np.concatenate([cosT, cosT], axis=0).astype(np.float16)  # (128, T)
    sin2 = np.concatenate([-sinT, sinT], axis=0).astype(np.float16)

    kk = np.arange(128)[:, None]
    qq = np.arange(512)[None, :]
    masks = np.concatenate(
        [(kk + 128 * d <= qq).astype(np.float32) for d in range(4)], axis=1
    ).astype(bf16)  # (128, 2048)

    in_maps = []
    for c in range(N_CORES):
        qrows = slice(NREP * HD * c, NREP * HD * (c + 1))
        krows = slice(HD * c, HD * (c + 1))
        in_maps.append({
            "xT": xT,
            "cos2": cos2,
            "sin2": sin2,
            "masks": masks,
            "wq_p": _pack_w(wq[qrows, :].T.astype(np.float16), NREP * HD),
            "wk_p": _pack_w(wk[krows, :].T.astype(np.float16), HD),
            "wv_p": _pack_w(wv[krows, :].T.astype(np.float16), HD),
            "wo_p": _pack_w(wo[qrows, :].T.astype(np.float16), NREP * HD),
        })
    return in_maps


def kernel(x, cos, sin, wq, wk, wv, wo, start_pos):
    in_maps = _shard_inputs(x, cos, sin, wq, wk, wv, wo, start_pos)
    nc = _build_kernel()
    res = run_bass_kernel_spmd(nc, in_maps, core_ids=list(range(N_CORES)))
    out = np.concatenate([res.results[c]["out"] for c in range(N_CORES)], axis=1)
    return out.reshape(1, T, DIM).astype(np.float32)
